# revision 70
# baseline (speedup 1.0000x reference)
"""Trainium2 8-core kernel for nn_CAT_81269371175150 (GNN message passing).

Math (see reference):
  gcn(x)   = selu(A_gn @ (x @ W1^T))            for features and aug_features
  S        = softmax_K(gcn1 @ Wt^T)
  loss     = spectral(S, A) + cluster(S) + 0.5 * con(gcn1, gcn2)

Strategy (fp8 pipeline, ~2.8x over the bf16 v2 baseline):
  * Nodes sharded row-wise across 8 cores; edge list bucketed by destination
    shard, sorted by (dest block, global src half, src col), padded to fixed
    chunk counts so all cores run one SPMD program.
  * Phase A: h1|h2 = [X|Xa] @ W1^T per shard with fp8 inputs x bf16 weights
    (three block-aligned passes for SBUF tiling, loads split across the
    sync+scalar HWDGE rings), cast to fp8 512B combined table rows.
  * ONE AllGather of the whole shard (a single collective pays the mesh
    barrier protocol once); gathers address the two table halves via base
    offsets so indices stay within int16.
  * SpMM: dma_gather of fp8 512B rows (4 SWDGE queues, 64KB descriptor
    carveout, per-core true counts in a register + negative-index padding
    skip) + one fp8 matmul per 128-edge chunk (one-hot lhsT with gn folded
    in, host-built, streamed on the scalar ring only after the AllGather)
    accumulating [A@h1 | A@h2] into one PSUM bank per destination block.
  * GT_LOOK=2: the gather stream barely runs ahead, so the AllGather data
    phase is never starved of SDMA engines.
  * selu runs mostly on ScalarE; S = softmax_K per block; log-softmax
    stats emitted per column segment as blocks complete (keeps them off
    the tail); con-loss partials accumulate on-chip.
  * Host finishes the tiny reductions: trace(S^T A S), nl, cluster sizes,
    log-softmax renormalization across 3 segments x 8 cores, final scalar.
"""

import math
import numpy as np
import ml_dtypes

import concourse.bacc as bacc
import concourse.mybir as mybir
import concourse.tile as tile
from concourse import bass_utils
from concourse.masks import make_identity

P = 128
NC = 8

# full-size problem constants
FULL = dict(N=50000, F=500, D=256, K=16)

SELU_L = 1.0507009873554805
SELU_A = 1.6732632423543772
SELU_LA = SELU_L * SELU_A
LN_SELU_LA = math.log(SELU_LA)

CLUSTER_REG = 1.0
CON_REG = 0.5

bf16 = mybir.dt.bfloat16
fp8 = mybir.dt.float8e4
f32 = mybir.dt.float32
i16 = mybir.dt.int16
i32 = mybir.dt.int32

GT_BUFS = 14   # gather tile ring depth; first ring pass gathers full
               # padded groups so every buffer byte is initialized before
               # negative-index skipping kicks in
GT_LOOK = 2   # h0 lookahead before the first h1 gather. Kept tiny so the
              # lookahead drain does not steal SDMA engines from AllGather-1
              # (must stay below GT_BUFS to avoid a buffer-reuse deadlock)


def cdiv(a, b):
    return -(-a // b)


# --------------------------------------------------------------------------
# host-side preprocessing
# --------------------------------------------------------------------------

def prep(features, aug_features, graph_row, graph_col, gn_vals, W1, Wt, cfg):
    N, F, D, K = cfg["N"], cfg["F"], cfg["D"], cfg["K"]
    NSH = N // NC
    NB = cdiv(NSH, P)
    HALF = N // 2

    row = np.asarray(graph_row).astype(np.int64)
    col = np.asarray(graph_col).astype(np.int64)
    gn = np.asarray(gn_vals).astype(np.float64)

    SEG = NSH // 2
    core = row // NSH
    per_core = []
    cnts = np.zeros((NC, NB, 2), dtype=np.int64)
    for c in range(NC):
        m = core == c
        r = row[m] - c * NSH
        cl = col[m]
        g = gn[m]
        b = r // P
        h = cl // HALF          # global table half (keeps idx < 32768/int16)
        order = np.lexsort((cl, h, b))
        r, cl, g, b, h = r[order], cl[order], g[order], b[order], h[order]
        key = b * 2 + h
        cnt = np.bincount(key, minlength=NB * 2).reshape(NB, 2)
        cnts[c] = cnt
        per_core.append((r, cl, g, b, h, key))

    CBH = np.ceil(cnts.max(axis=0) / P).astype(np.int64)        # [NB, 2]
    nch_b = CBH[:, 0] + CBH[:, 1]
    NCHT = int(nch_b.sum())
    strm_base = np.zeros((2, NB), dtype=np.int64)
    for h in range(2):
        strm_base[h] = np.concatenate([[0], np.cumsum(CBH[:, h])[:-1]])
    Lh = [int(CBH[:, h].sum()) * P for h in range(2)]
    ohbase = np.concatenate([[0], np.cumsum(nch_b)[:-1]])

    X = np.asarray(features)[0]
    Xa = np.asarray(aug_features)[0]
    XT = np.ascontiguousarray(X.T).astype(ml_dtypes.float8_e4m3)   # [F, N]
    XTa = np.ascontiguousarray(Xa.T).astype(ml_dtypes.float8_e4m3)
    W1T = np.ascontiguousarray(np.asarray(W1).T).astype(ml_dtypes.bfloat16)
    WtT = np.ascontiguousarray(np.asarray(Wt).T).astype(ml_dtypes.bfloat16)

    def wrap_idx(a):
        # [L] -> [128, L/16]: element i at [i%16, i//16], replicated x8
        L = a.shape[0]
        w = a.reshape(L // 16, 16).T
        return np.ascontiguousarray(np.tile(w, (8, 1)))

    in_maps = []
    for c in range(NC):
        r, cl, g, b, h, key = per_core[c]
        cnt = cnts[c]
        run_start = np.zeros(NB * 2, dtype=np.int64)
        flat = cnt.reshape(-1)
        run_start[1:] = np.cumsum(flat)[:-1]
        rank = np.arange(len(r)) - run_start[key]
        lane = rank % P
        j = rank // P

        idx_streams = []
        gcnt = np.zeros(2 * NB, dtype=np.int32)
        loc = cl - h * HALF
        emit_rank = {}
        rk = 0
        for bb in range(min(GT_LOOK, NB)):
            emit_rank[(0, bb)] = rk
            rk += 1
        for bb in range(NB):
            emit_rank[(1, bb)] = rk
            rk += 1
            if bb + GT_LOOK < NB:
                emit_rank[(0, bb + GT_LOOK)] = rk
                rk += 1
        for hh in range(2):
            arr = np.full(Lh[hh], -1, dtype=np.int16)
            m = h == hh
            off = (strm_base[hh][b[m]] + j[m]) * P + lane[m]
            arr[off] = loc[m].astype(np.int16)
            for bb in range(NB):
                n = CBH[bb][hh]
                if n == 0:
                    gcnt[hh * NB + bb] = 0
                    continue
                base = strm_base[hh][bb] * P
                cv = int(cnt[bb][hh])
                if emit_rank.get((hh, bb), 0) < GT_BUFS:
                    eff = n * P          # first ring pass: gather everything
                else:
                    eff = min(max(cdiv(cv, 16) * 16, P), n * P)
                arr[base + cv:base + eff] = 0
                gcnt[hh * NB + bb] = eff
            idx_streams.append(wrap_idx(arr))

        oh = np.zeros((P, NCHT, P), dtype=ml_dtypes.float8_e4m3)
        ohcol = ohbase[b] + h * CBH[b, 0] + j
        dest = r - b * P
        oh[lane, ohcol, dest] = g.astype(ml_dtypes.float8_e4m3)

        in_maps.append({
            "xt": np.ascontiguousarray(XT[:, c * NSH:(c + 1) * NSH]),
            "xta": np.ascontiguousarray(XTa[:, c * NSH:(c + 1) * NSH]),
            "w1t": W1T,
            "wtt": WtT,
            "oh": oh,
            "idx0": idx_streams[0],
            "idx1": idx_streams[1],
            "gcnt": gcnt.reshape(1, -1),
        })

    meta = dict(
        N=N, F=F, D=D, K=K, NSH=NSH, NB=NB, SEG=SEG, DT=D // P,
        HALF=HALF,
        CBH=tuple(map(tuple, CBH.tolist())), NCHT=NCHT,
        strm_base=tuple(map(tuple, strm_base.tolist())),
        Lh=tuple(Lh), ohbase=tuple(ohbase.tolist()),
        FT=cdiv(F, P),
    )
    return in_maps, meta


# --------------------------------------------------------------------------
# device program
# --------------------------------------------------------------------------

def build(meta, debug=False):
    N, F, D, K = meta["N"], meta["F"], meta["D"], meta["K"]
    NSH, NB, SEG, DT = meta["NSH"], meta["NB"], meta["SEG"], meta["DT"]
    CBH = meta["CBH"]
    NCHT = meta["NCHT"]
    strm_base = meta["strm_base"]
    Lh = meta["Lh"]
    ohbase = meta["ohbase"]
    FT = meta["FT"]
    W2 = 2 * D                  # combined table row elems (bf16)
    VLEN = 2 * D

    nc = bacc.Bacc("TRN2", target_bir_lowering=False, debug=debug,
                   num_devices=NC, num_swdge_queues=4,
                   dynamic_dma_scratch_size=65536)

    xt = nc.dram_tensor("xt", [F, NSH], fp8, kind="ExternalInput")
    xta = nc.dram_tensor("xta", [F, NSH], fp8, kind="ExternalInput")
    w1t = nc.dram_tensor("w1t", [F, D], bf16, kind="ExternalInput")
    wtt = nc.dram_tensor("wtt", [D, K], bf16, kind="ExternalInput")
    oh = nc.dram_tensor("oh", [P, NCHT, P], fp8, kind="ExternalInput")
    idx_d = [nc.dram_tensor(f"idx{h}", [P, Lh[h] // 16], i16, kind="ExternalInput")
             for h in range(2)]
    gcnt_d = nc.dram_tensor("gcnt", [1, 2 * NB], i32, kind="ExternalInput")

    stats_p_d = nc.dram_tensor("stats_p", [P, 6 * DT], f32, kind="ExternalOutput")
    stats_v_d = nc.dram_tensor("stats_v", [1, VLEN], f32, kind="ExternalOutput")
    s_out_d = nc.dram_tensor("s_out", [NB * P, K], f32, kind="ExternalOutput")

    max_nch_b = max(CBH[b][0] + CBH[b][1] for b in range(NB))
    max_grp = max((CBH[b][h] for b in range(NB) for h in range(2)), default=1)

    with tile.TileContext(nc) as tc:
        with (
            tc.tile_pool(name="big", bufs=12) as bigp,
            tc.tile_pool(name="exp", bufs=2) as expp,
            tc.tile_pool(name="gtp", bufs=14) as gtp,
            tc.tile_pool(name="ohp", bufs=6) as ohp,
            tc.tile_pool(name="persist", bufs=1) as persist,
            tc.tile_pool(name="stage", bufs=3) as stagep,
            tc.tile_pool(name="tmp", bufs=2) as tmpp,
            tc.tile_pool(name="small", bufs=4) as smallp,
            tc.tile_pool(name="svp", bufs=1) as svp,
            tc.tile_pool(name="pa", bufs=5, space="PSUM") as pa,
            tc.tile_pool(name="pb", bufs=3, space="PSUM") as pb,
            tc.tile_pool(name="dram", bufs=1, space="DRAM") as dramp,
        ):
            # ---- constants / resident tensors
            ident = persist.tile([P, P], f32)
            make_identity(nc, ident[:])
            w1t_t = persist.tile([P, FT, D], bf16)
            for t in range(FT):
                fr = min(P, F - t * P)
                nc.sync.dma_start(w1t_t[:fr, t, :], w1t[t * P:t * P + fr, :])
            wtt_t = persist.tile([P, DT, K], bf16)
            for t in range(DT):
                nc.sync.dma_start(wtt_t[:, t, :], wtt[t * P:(t + 1) * P, :])
            idx_t = []
            for h in range(2):
                it = persist.tile([P, Lh[h] // 16], i16, tag=f"idx{h}")
                nc.sync.dma_start(it[:], idx_d[h][:])
                idx_t.append(it)
            gcnt_t = persist.tile([1, 2 * NB], i32, tag="gcnt")
            nc.sync.dma_start(gcnt_t[:], gcnt_d[:])

            ln_la = persist.tile([P, 1], f32, tag="lnla")
            nc.vector.memset(ln_la[:], LN_SELU_LA)
            la_c = persist.tile([P, 1], f32, tag="lac")
            nc.vector.memset(la_c[:], SELU_LA)

            gcn1T = persist.tile([P, DT, NB * P], fp8, tag="gcn1T")
            accs = persist.tile([P, VLEN], f32, tag="accs")
            nc.vector.memset(accs[:], 0.0)
            stats_p = persist.tile([P, 6 * DT], f32, tag="statsp")

            HALF = meta["HALF"]
            cc_in = dramp.tile([NSH, W2], fp8, name="cc_in")
            cc_out = dramp.tile([N, W2], fp8, addr_space="Shared",
                                name="cc_out")

            # ================= phase A: h1|h2 = [X|Xa] @ W1^T =============
            # three block-aligned passes purely for SBUF tiling (pool holds
            # one pass + one pass of prefetch); one AllGather of the whole
            # shard afterwards (single collective = single barrier-protocol
            # cost; gathers use per-half base offsets to stay within int16
            # index range).
            NB0 = cdiv(SEG, P)
            for q, (b_lo, b_hi) in enumerate(((0, 17), (17, 33), (33, NB))):
                c0 = b_lo * P
                c1 = min(b_hi * P, NSH)
                W = c1 - c0
                xt_tiles = []
                for which, src in enumerate((xt, xta)):
                    tl = []
                    for t in range(FT):
                        fr = min(P, F - t * P)
                        xx = bigp.tile([P, W], fp8, tag="big",
                                       name=f"xx{q}_{which}_{t}")
                        eng = nc.sync if (which * FT + t) % 2 == 0 else nc.scalar
                        eng.dma_start(xx[:fr, :], src[t * P:t * P + fr, c0:c1])
                        tl.append(xx)
                    xt_tiles.append(tl)
                for b in range(b_lo, b_hi):
                    rows = min(P, NSH - b * P)
                    off = b * P - c0
                    ptw = pb.tile([P, W2], f32, space="PSUM", tag="pb")
                    for which in range(2):
                        for t in range(FT):
                            fr = min(P, F - t * P)
                            nc.tensor.matmul(
                                ptw[:rows, which * D:(which + 1) * D],
                                lhsT=xt_tiles[which][t][:fr, off:off + rows],
                                rhs=w1t_t[:fr, t, :],
                                start=(t == 0), stop=(t == FT - 1),
                            )
                    st = stagep.tile([P, W2], fp8, tag="stage")
                    nc.vector.tensor_copy(st[:rows, :], ptw[:rows, :])
                    nc.scalar.dma_start(cc_in[b * P:b * P + rows, :],
                                        st[:rows, :])
            nc.gpsimd.collective_compute(
                "AllGather", mybir.AluOpType.bypass,
                replica_groups=[list(range(NC))],
                ins=[cc_in[:]], outs=[cc_out[:]],
            )

            # ================= phase B: fused SpMM + epilogues =============
            def selu_into(dst_ap, psum_ap):
                """dst = selu(psum), mostly on ScalarE.

                e2 = exp(x + ln(la));  e3 = relu(la - e2);  r = relu(l*x)
                selu = r - e3
                """
                e2 = tmpp.tile([P, D], f32, tag="tmpd")
                nc.scalar.activation(e2[:], psum_ap,
                                     mybir.ActivationFunctionType.Exp,
                                     bias=ln_la[:])
                e3 = tmpp.tile([P, D], f32, tag="tmpd2")
                nc.scalar.activation(e3[:], e2[:],
                                     mybir.ActivationFunctionType.Relu,
                                     bias=la_c[:], scale=-1.0)
                r = tmpp.tile([P, D], f32, tag="tmpd3")
                nc.scalar.activation(r[:], psum_ap,
                                     mybir.ActivationFunctionType.Relu,
                                     scale=SELU_L)
                nc.vector.tensor_tensor(dst_ap, r[:], e3[:],
                                        mybir.AluOpType.subtract)

            gtile = {}
            LOOK = GT_LOOK
            gorder = [(0, b) for b in range(min(LOOK, NB))]
            for b in range(NB):
                gorder.append((1, b))
                if b + LOOK < NB:
                    gorder.append((0, b + LOOK))
            prev_inst = None
            gq = 0
            cnt_reg = nc.gpsimd.alloc_register("gcnt_reg")
            g10_inst = None
            for emit_rank, (h, g) in enumerate(gorder):
                sc = strm_base[h][g]
                n = CBH[g][h]
                gt = gtp.tile([P, max_grp, W2], fp8, tag="gt",
                              name=f"gt_{h}_{g}")
                if n > 0:
                    if emit_rank < GT_BUFS and n < max_grp:
                        # first ring pass leaves slices >= n uninitialized;
                        # zero them so later pad lanes never read NaN bits
                        nc.vector.memset(gt[:, n:max_grp, :], 0.0)
                    nidx = n * P
                    ld = nc.gpsimd.reg_load(
                        cnt_reg, gcnt_t[0:1, h * NB + g:h * NB + g + 1])
                    if prev_inst is not None:
                        tile.add_dep_helper(ld.ins, prev_inst, sync=False,
                                            reason="gather issue order")
                    gi = nc.gpsimd.dma_gather(
                        gt[:, 0:n, :], cc_out[h * HALF:(h + 1) * HALF, :],
                        idx_t[h][:, sc * 8:(sc + n) * 8],
                        num_idxs=nidx, num_idxs_reg=cnt_reg, elem_size=W2,
                        single_packet=False,
                        queue_num=gq % 4,
                    )
                    gq += 1
                    tile.add_dep_helper(gi.ins, ld.ins, sync=False,
                                        reason="count reg load order")
                    prev_inst = gi.ins
                    if (h, g) == (1, 0):
                        g10_inst = gi.ins
                gtile[(h, g)] = (gt, sc)

            def get_gtile(h, g):
                return gtile[(h, g)]

            first_oh = True
            for b in range(NB):
                rows = min(P, NSH - b * P)
                nch = CBH[b][0] + CBH[b][1]
                oht = ohp.tile([P, max_nch_b, P], fp8, tag="oh")
                ohl = nc.scalar.dma_start(oht[:, 0:nch, :],
                                          oh[:, ohbase[b]:ohbase[b] + nch, :])
                if first_oh:
                    # keep the 14MB one-hot stream off the wires until both
                    # AllGathers are done (block-0 matmuls need AG1 anyway)
                    tile.add_dep_helper(ohl.ins, g10_inst, sync=True,
                                        reason="delay oh stream")
                    first_oh = False
                pt = pa.tile([P, W2], f32, space="PSUM", tag="pa")
                nmm = 0
                tot = CBH[b][0] + CBH[b][1]
                if tot == 0:
                    nc.vector.memset(pt[:], 0.0)
                for h in range(2):
                    n = CBH[b][h]
                    if n == 0:
                        continue
                    gt, s = get_gtile(h, b)
                    base_oh = h * CBH[b][0]
                    for j in range(n):
                        c = strm_base[h][b] + j - s
                        nc.tensor.matmul(
                            pt[:], lhsT=oht[:, base_oh + j, :],
                            rhs=gt[:, c, :],
                            start=(nmm == 0), stop=(nmm == tot - 1))
                        nmm += 1

                # epilogue
                g1b = tmpp.tile([P, D], f32, tag="g1b")
                selu_into(g1b[:], pt[:, 0:D])
                aug = tmpp.tile([P, D], f32, tag="aug")
                selu_into(aug[:], pt[:, D:W2])
                # con partials
                nc.vector.tensor_tensor(accs[:, 0:D], accs[:, 0:D], aug[:],
                                        mybir.AluOpType.add)
                pr = tmpp.tile([P, D], f32, tag="pr")
                nc.vector.tensor_tensor(pr[:], aug[:], g1b[:],
                                        mybir.AluOpType.mult)
                nc.vector.tensor_tensor(accs[:, D:W2], accs[:, D:W2], pr[:],
                                        mybir.AluOpType.add)
                for t in range(DT):
                    ptr = pb.tile([P, P], f32, space="PSUM", tag="pb")
                    nc.tensor.transpose(ptr[:], g1b[:, t * P:(t + 1) * P],
                                        ident[:])
                    nc.vector.tensor_copy(gcn1T[:, t, b * P:(b + 1) * P], ptr[:])
                pl = pb.tile([P, K], f32, space="PSUM", tag="pb")
                for t in range(DT):
                    nc.tensor.matmul(pl[:], lhsT=gcn1T[:, t, b * P:(b + 1) * P],
                                     rhs=wtt_t[:, t, :],
                                     start=(t == 0), stop=(t == DT - 1))
                nmx = smallp.tile([P, 1], f32, tag="nmx")
                nc.vector.reduce_max(nmx[:], pl[:], axis=mybir.AxisListType.X,
                                     negate=True)
                ex = smallp.tile([P, K], f32, tag="ex")
                sm = smallp.tile([P, 1], f32, tag="sm")
                nc.scalar.activation(ex[:], pl[:],
                                     mybir.ActivationFunctionType.Exp,
                                     bias=nmx[:], accum_out=sm[:])
                rc = smallp.tile([P, 1], f32, tag="rc")
                nc.vector.reciprocal(rc[:], sm[:])
                sb = stagep.tile([P, K], f32, tag="sstage")
                nc.scalar.mul(sb[:], ex[:], rc[:])
                nc.sync.dma_start(s_out_d[b * P:(b + 1) * P, :], sb[:])

                # log-softmax stats for a column segment as soon as its
                # last block's gcn1T slice lands (keeps them off the tail)
                seg_ends = {NB0 - 1: (0, 0, NB0 * P),
                            NB - 2: (1, NB0 * P, (NB - 1) * P),
                            NB - 1: (2, (NB - 1) * P, NSH)}
                if b in seg_ends:
                    si, cl_, cr_ = seg_ends[b]
                    for t in range(DT):
                        nmt = smallp.tile([P, 1], f32, tag="nmt")
                        nc.vector.reduce_max(nmt[:], gcn1T[:, t, cl_:cr_],
                                             axis=mybir.AxisListType.X,
                                             negate=True)
                        nc.scalar.mul(
                            stats_p[:, si * 2 * DT + t:si * 2 * DT + t + 1],
                            nmt[:], -1.0)
                        sacc = stats_p[:, si * 2 * DT + DT + t:
                                       si * 2 * DT + DT + t + 1]
                        if cr_ - cl_ <= 17 * P:
                            exb = expp.tile([P, 17 * P], bf16, tag="exp")
                            nc.scalar.activation(
                                exb[:, 0:cr_ - cl_], gcn1T[:, t, cl_:cr_],
                                mybir.ActivationFunctionType.Exp, bias=nmt[:],
                                accum_out=sacc)
                        else:
                            mid = (cl_ + cr_) // 2
                            parts = smallp.tile([P, 2], f32, tag="parts")
                            for ci, (e0, e1) in enumerate(((cl_, mid),
                                                          (mid, cr_))):
                                exb = expp.tile([P, 17 * P], bf16, tag="exp")
                                nc.scalar.activation(
                                    exb[:, 0:e1 - e0], gcn1T[:, t, e0:e1],
                                    mybir.ActivationFunctionType.Exp,
                                    bias=nmt[:], accum_out=parts[:, ci:ci + 1])
                            nc.vector.tensor_tensor(sacc, parts[:, 0:1],
                                                    parts[:, 1:2],
                                                    mybir.AluOpType.add)

            ones = persist.tile([P, 1], f32, tag="ones")
            nc.vector.memset(ones[:], 1.0)
            pv = pa.tile([P, VLEN], f32, space="PSUM", tag="pa")
            nc.tensor.matmul(pv[0:1, :], lhsT=ones[:], rhs=accs[:],
                             start=True, stop=True)
            sv = svp.tile([1, VLEN], f32, tag="sv")
            nc.vector.tensor_copy(sv[:], pv[0:1, :])
            nc.sync.dma_start(stats_v_d[:], sv[:])
            nc.sync.dma_start(stats_p_d[:], stats_p[:])

    nc.compile()
    return nc


# --------------------------------------------------------------------------
# host-side combine of per-core partials
# --------------------------------------------------------------------------

def combine(results, cfg, graph_row, graph_col, graph_vals):
    N, D, K = cfg["N"], cfg["D"], cfg["K"]
    NSH = N // NC
    DT = D // P
    E = float(graph_row.shape[0])

    m, s = [], []
    colsum_aug = np.zeros(D)
    dot = 0.0
    S_full = np.zeros((N, K))
    for c in range(NC):
        sp = np.asarray(results[c]["stats_p"], dtype=np.float64)
        svv = np.asarray(results[c]["stats_v"], dtype=np.float64).reshape(-1)
        for si in range(3):
            m.append(np.concatenate(
                [sp[:, si * 2 * DT + t] for t in range(DT)]))
            s.append(np.concatenate(
                [sp[:, si * 2 * DT + DT + t] for t in range(DT)]))
        colsum_aug += svv[0:D]
        dot += svv[D:2 * D].sum()
        S_full[c * NSH:(c + 1) * NSH] = \
            np.asarray(results[c]["s_out"], dtype=np.float64)[:NSH]
    m = np.stack(m)
    s = np.stack(s)
    M = m.max(axis=0)
    Sg = (np.exp(m - M) * s).sum(axis=0)
    logZ = M + np.log(Sg)

    row = np.asarray(graph_row).astype(np.int64)
    col = np.asarray(graph_col).astype(np.int64)
    av = np.asarray(graph_vals).astype(np.float64)
    deg = np.bincount(col, weights=av, minlength=N).astype(np.float64)

    trace_gp = np.einsum('e,ek,ek->', av, S_full[row], S_full[col])
    nl = S_full.T @ deg
    clsz = S_full.sum(axis=0)

    spectral = -(trace_gp - (nl ** 2).sum() / (2.0 * E)) / (2.0 * E)
    cluster = (np.linalg.norm(clsz) / N * math.sqrt(K) - 1.0) * CLUSTER_REG
    con = -(dot - (logZ * colsum_aug).sum()) / D
    return spectral + cluster + CON_REG * con


# --------------------------------------------------------------------------
# entry point
# --------------------------------------------------------------------------

_BUILD_CACHE = {}


def kernel(features, aug_features, graph_row, graph_col, graph_vals, gn_vals,
           lbl, dense_graph, W1, b1, Wt, bt, _cfg=None, _trace=False):
    cfg = _cfg or FULL
    in_maps, meta = prep(features, aug_features, graph_row, graph_col,
                         gn_vals, W1, Wt, cfg)
    key = tuple(sorted((k, str(v)) for k, v in meta.items()))
    if key not in _BUILD_CACHE:
        _BUILD_CACHE[key] = build(meta)
    nc = _BUILD_CACHE[key]
    res = bass_utils.run_bass_kernel_spmd(nc, in_maps, core_ids=list(range(NC)),
                                          trace=_trace)
    loss = combine(res.results, cfg, graph_row, graph_col, graph_vals)
    out = np.array(loss, dtype=np.float32)
    if _trace:
        return out, res
    return out



# revision 71
# speedup vs baseline: 1.0533x; 1.0533x over previous
"""Trainium2 8-core kernel for nn_CAT_81269371175150 (GNN message passing).

Math (see reference):
  gcn(x)   = selu(A_gn @ (x @ W1^T))            for features and aug_features
  S        = softmax_K(gcn1 @ Wt^T)
  loss     = spectral(S, A) + cluster(S) + 0.5 * con(gcn1, gcn2)

Strategy (fp8 pipeline, ~2.8x over the bf16 v2 baseline):
  * Nodes sharded row-wise across 8 cores; edge list bucketed by destination
    shard, sorted by (dest block, global src half, src col), padded to fixed
    chunk counts so all cores run one SPMD program.
  * Phase A: h1|h2 = [X|Xa] @ W1^T per shard with fp8 inputs x bf16 weights
    (three block-aligned passes for SBUF tiling, loads split across the
    sync+scalar HWDGE rings), cast to fp8 512B combined table rows.
  * ONE AllGather of the whole shard (a single collective pays the mesh
    barrier protocol once); gathers address the two table halves via base
    offsets so indices stay within int16.
  * SpMM: dma_gather of fp8 512B rows (4 SWDGE queues, 64KB descriptor
    carveout, per-core true counts in a register + negative-index padding
    skip) + one fp8 matmul per 128-edge chunk (one-hot lhsT with gn folded
    in, host-built, streamed on the scalar ring only after the AllGather)
    accumulating [A@h1 | A@h2] into one PSUM bank per destination block.
  * GT_LOOK=2: the gather stream barely runs ahead, so the AllGather data
    phase is never starved of SDMA engines.
  * selu runs mostly on ScalarE; S = softmax_K per block; log-softmax
    stats emitted per column segment as blocks complete (keeps them off
    the tail); con-loss partials accumulate on-chip.
  * Host finishes the tiny reductions: trace(S^T A S), nl, cluster sizes,
    log-softmax renormalization across 3 segments x 8 cores, final scalar.
"""

import math
import numpy as np
import ml_dtypes

import concourse.bacc as bacc
import concourse.mybir as mybir
import concourse.tile as tile
from concourse import bass_utils
from concourse.masks import make_identity

P = 128
NC = 8

# full-size problem constants
FULL = dict(N=50000, F=500, D=256, K=16)

SELU_L = 1.0507009873554805
SELU_A = 1.6732632423543772
SELU_LA = SELU_L * SELU_A
LN_SELU_LA = math.log(SELU_LA)

CLUSTER_REG = 1.0
CON_REG = 0.5

bf16 = mybir.dt.bfloat16
fp8 = mybir.dt.float8e4
f32 = mybir.dt.float32
i16 = mybir.dt.int16
i32 = mybir.dt.int32

GT_BUFS = 11   # gather tile ring depth; first ring pass gathers full
               # padded groups so every buffer byte is initialized before
               # negative-index skipping kicks in
GT_LOOK = 2   # h0 lookahead before the first h1 gather. Kept tiny so the
              # lookahead drain does not steal SDMA engines from AllGather-1
              # (must stay below GT_BUFS to avoid a buffer-reuse deadlock)


def cdiv(a, b):
    return -(-a // b)


# --------------------------------------------------------------------------
# host-side preprocessing
# --------------------------------------------------------------------------

def prep(features, aug_features, graph_row, graph_col, gn_vals, W1, Wt, cfg):
    N, F, D, K = cfg["N"], cfg["F"], cfg["D"], cfg["K"]
    NSH = N // NC
    NB = cdiv(NSH, P)
    HALF = N // 2

    row = np.asarray(graph_row).astype(np.int64)
    col = np.asarray(graph_col).astype(np.int64)
    gn = np.asarray(gn_vals).astype(np.float64)

    SEG = NSH // 2
    core = row // NSH
    per_core = []
    cnts = np.zeros((NC, NB, 2), dtype=np.int64)
    for c in range(NC):
        m = core == c
        r = row[m] - c * NSH
        cl = col[m]
        g = gn[m]
        b = r // P
        h = cl // HALF          # global table half (keeps idx < 32768/int16)
        order = np.lexsort((cl, h, b))
        r, cl, g, b, h = r[order], cl[order], g[order], b[order], h[order]
        key = b * 2 + h
        cnt = np.bincount(key, minlength=NB * 2).reshape(NB, 2)
        cnts[c] = cnt
        per_core.append((r, cl, g, b, h, key))

    CBH = np.ceil(cnts.max(axis=0) / P).astype(np.int64)        # [NB, 2]
    nch_b = CBH[:, 0] + CBH[:, 1]
    NCHT = int(nch_b.sum())
    strm_base = np.zeros((2, NB), dtype=np.int64)
    for h in range(2):
        strm_base[h] = np.concatenate([[0], np.cumsum(CBH[:, h])[:-1]])
    Lh = [int(CBH[:, h].sum()) * P for h in range(2)]
    ohbase = np.concatenate([[0], np.cumsum(nch_b)[:-1]])

    X = np.asarray(features)[0]
    Xa = np.asarray(aug_features)[0]
    XT = np.ascontiguousarray(X.T).astype(ml_dtypes.float8_e4m3)   # [F, N]
    XTa = np.ascontiguousarray(Xa.T).astype(ml_dtypes.float8_e4m3)
    W1T = np.ascontiguousarray(np.asarray(W1).T).astype(ml_dtypes.bfloat16)
    WtT = np.ascontiguousarray(np.asarray(Wt).T).astype(ml_dtypes.bfloat16)

    def wrap_idx(a):
        # [L] -> [128, L/16]: element i at [i%16, i//16], replicated x8
        L = a.shape[0]
        w = a.reshape(L // 16, 16).T
        return np.ascontiguousarray(np.tile(w, (8, 1)))

    in_maps = []
    for c in range(NC):
        r, cl, g, b, h, key = per_core[c]
        cnt = cnts[c]
        run_start = np.zeros(NB * 2, dtype=np.int64)
        flat = cnt.reshape(-1)
        run_start[1:] = np.cumsum(flat)[:-1]
        rank = np.arange(len(r)) - run_start[key]
        lane = rank % P
        j = rank // P

        idx_streams = []
        gcnt = np.zeros(2 * NB, dtype=np.int32)
        loc = cl - h * HALF
        emit_rank = {}
        rk = 0
        for bb in range(min(GT_LOOK, NB)):
            emit_rank[(0, bb)] = rk
            rk += 1
        for bb in range(NB):
            emit_rank[(1, bb)] = rk
            rk += 1
            if bb + GT_LOOK < NB:
                emit_rank[(0, bb + GT_LOOK)] = rk
                rk += 1
        for hh in range(2):
            arr = np.full(Lh[hh], -1, dtype=np.int16)
            m = h == hh
            off = (strm_base[hh][b[m]] + j[m]) * P + lane[m]
            arr[off] = loc[m].astype(np.int16)
            for bb in range(NB):
                n = CBH[bb][hh]
                if n == 0:
                    gcnt[hh * NB + bb] = 0
                    continue
                base = strm_base[hh][bb] * P
                cv = int(cnt[bb][hh])
                if emit_rank.get((hh, bb), 0) < GT_BUFS:
                    eff = n * P          # first ring pass: gather everything
                else:
                    eff = min(max(cdiv(cv, 16) * 16, P), n * P)
                arr[base + cv:base + eff] = 0
                gcnt[hh * NB + bb] = eff
            idx_streams.append(wrap_idx(arr))

        oh = np.zeros((P, NCHT, P), dtype=ml_dtypes.float8_e4m3)
        ohcol = ohbase[b] + h * CBH[b, 0] + j
        dest = r - b * P
        oh[lane, ohcol, dest] = g.astype(ml_dtypes.float8_e4m3)

        in_maps.append({
            "xt": np.ascontiguousarray(XT[:, c * NSH:(c + 1) * NSH]),
            "xta": np.ascontiguousarray(XTa[:, c * NSH:(c + 1) * NSH]),
            "w1t": W1T,
            "wtt": WtT,
            "oh": oh,
            "idx0": idx_streams[0],
            "idx1": idx_streams[1],
            "gcnt": gcnt.reshape(1, -1),
        })

    meta = dict(
        N=N, F=F, D=D, K=K, NSH=NSH, NB=NB, SEG=SEG, DT=D // P,
        HALF=HALF,
        CBH=tuple(map(tuple, CBH.tolist())), NCHT=NCHT,
        strm_base=tuple(map(tuple, strm_base.tolist())),
        Lh=tuple(Lh), ohbase=tuple(ohbase.tolist()),
        FT=cdiv(F, P),
    )
    return in_maps, meta


# --------------------------------------------------------------------------
# device program
# --------------------------------------------------------------------------

def build(meta, debug=False):
    N, F, D, K = meta["N"], meta["F"], meta["D"], meta["K"]
    NSH, NB, SEG, DT = meta["NSH"], meta["NB"], meta["SEG"], meta["DT"]
    CBH = meta["CBH"]
    NCHT = meta["NCHT"]
    strm_base = meta["strm_base"]
    Lh = meta["Lh"]
    ohbase = meta["ohbase"]
    FT = meta["FT"]
    W2 = 2 * D                  # combined table row elems (bf16)
    VLEN = 2 * D

    nc = bacc.Bacc("TRN2", target_bir_lowering=False, debug=debug,
                   num_devices=NC, num_swdge_queues=4,
                   dynamic_dma_scratch_size=65536)

    xt = nc.dram_tensor("xt", [F, NSH], fp8, kind="ExternalInput")
    xta = nc.dram_tensor("xta", [F, NSH], fp8, kind="ExternalInput")
    w1t = nc.dram_tensor("w1t", [F, D], bf16, kind="ExternalInput")
    wtt = nc.dram_tensor("wtt", [D, K], bf16, kind="ExternalInput")
    oh = nc.dram_tensor("oh", [P, NCHT, P], fp8, kind="ExternalInput")
    idx_d = [nc.dram_tensor(f"idx{h}", [P, Lh[h] // 16], i16, kind="ExternalInput")
             for h in range(2)]
    gcnt_d = nc.dram_tensor("gcnt", [1, 2 * NB], i32, kind="ExternalInput")

    stats_p_d = nc.dram_tensor("stats_p", [P, 6 * DT], f32, kind="ExternalOutput")
    stats_v_d = nc.dram_tensor("stats_v", [1, VLEN], f32, kind="ExternalOutput")
    s_out_d = nc.dram_tensor("s_out", [NB * P, K], f32, kind="ExternalOutput")

    max_nch_b = max(CBH[b][0] + CBH[b][1] for b in range(NB))
    max_grp = max((CBH[b][h] for b in range(NB) for h in range(2)), default=1)

    with tile.TileContext(nc) as tc:
        with (
            tc.tile_pool(name="big", bufs=12) as bigp,
            tc.tile_pool(name="gtp", bufs=11) as gtp,
            tc.tile_pool(name="ohp", bufs=5) as ohp,
            tc.tile_pool(name="persist", bufs=1) as persist,
            tc.tile_pool(name="stage", bufs=3) as stagep,
            tc.tile_pool(name="tmp", bufs=2) as tmpp,
            tc.tile_pool(name="small", bufs=4) as smallp,
            tc.tile_pool(name="svp", bufs=1) as svp,
            tc.tile_pool(name="pa", bufs=5, space="PSUM") as pa,
            tc.tile_pool(name="pb", bufs=3, space="PSUM") as pb,
            tc.tile_pool(name="dram", bufs=1, space="DRAM") as dramp,
        ):
            # ---- constants / resident tensors
            ident = persist.tile([P, P], f32)
            make_identity(nc, ident[:])
            w1t_t = persist.tile([P, FT, D], bf16)
            for t in range(FT):
                fr = min(P, F - t * P)
                nc.sync.dma_start(w1t_t[:fr, t, :], w1t[t * P:t * P + fr, :])
            wtt_t = persist.tile([P, DT, K], bf16)
            for t in range(DT):
                nc.sync.dma_start(wtt_t[:, t, :], wtt[t * P:(t + 1) * P, :])
            idx_t = []
            for h in range(2):
                it = persist.tile([P, Lh[h] // 16], i16, tag=f"idx{h}")
                nc.sync.dma_start(it[:], idx_d[h][:])
                idx_t.append(it)
            gcnt_t = persist.tile([1, 2 * NB], i32, tag="gcnt")
            nc.sync.dma_start(gcnt_t[:], gcnt_d[:])

            ln_la = persist.tile([P, 1], f32, tag="lnla")
            nc.vector.memset(ln_la[:], LN_SELU_LA)
            la_c = persist.tile([P, 1], f32, tag="lac")
            nc.vector.memset(la_c[:], SELU_LA)

            gcn1T = persist.tile([P, DT, NB * P], fp8, tag="gcn1T")
            accs = persist.tile([P, VLEN], f32, tag="accs")
            nc.vector.memset(accs[:], 0.0)
            stats_p = persist.tile([P, 6 * DT], f32, tag="statsp")

            HALF = meta["HALF"]
            cc_in = dramp.tile([NSH, W2], fp8, name="cc_in")
            cc_out = dramp.tile([N, W2], fp8, addr_space="Shared",
                                name="cc_out")

            # ================= phase A: h1|h2 = [X|Xa] @ W1^T =============
            # three block-aligned passes purely for SBUF tiling (pool holds
            # one pass + one pass of prefetch); one AllGather of the whole
            # shard afterwards (single collective = single barrier-protocol
            # cost; gathers use per-half base offsets to stay within int16
            # index range).
            NB0 = cdiv(SEG, P)
            for q, (b_lo, b_hi) in enumerate(((0, 17), (17, 33), (33, NB))):
                c0 = b_lo * P
                c1 = min(b_hi * P, NSH)
                W = c1 - c0
                xt_tiles = []
                for which, src in enumerate((xt, xta)):
                    tl = []
                    for t in range(FT):
                        fr = min(P, F - t * P)
                        xx = bigp.tile([P, W], fp8, tag="big",
                                       name=f"xx{q}_{which}_{t}")
                        eng = nc.sync if (which * FT + t) % 2 == 0 else nc.scalar
                        eng.dma_start(xx[:fr, :], src[t * P:t * P + fr, c0:c1])
                        tl.append(xx)
                    xt_tiles.append(tl)
                for b in range(b_lo, b_hi):
                    rows = min(P, NSH - b * P)
                    off = b * P - c0
                    ptw = pb.tile([P, W2], f32, space="PSUM", tag="pb")
                    for which in range(2):
                        for t in range(FT):
                            fr = min(P, F - t * P)
                            nc.tensor.matmul(
                                ptw[:rows, which * D:(which + 1) * D],
                                lhsT=xt_tiles[which][t][:fr, off:off + rows],
                                rhs=w1t_t[:fr, t, :],
                                start=(t == 0), stop=(t == FT - 1),
                            )
                    st = stagep.tile([P, W2], fp8, tag="stage")
                    nc.vector.tensor_copy(st[:rows, :], ptw[:rows, :])
                    nc.scalar.dma_start(cc_in[b * P:b * P + rows, :],
                                        st[:rows, :])
            nc.gpsimd.collective_compute(
                "AllGather", mybir.AluOpType.bypass,
                replica_groups=[list(range(NC))],
                ins=[cc_in[:]], outs=[cc_out[:]],
            )

            # ================= phase B: fused SpMM + epilogues =============
            def selu_into(dst_ap, psum_ap):
                """dst = selu(psum), mostly on ScalarE.

                e2 = exp(x + ln(la));  e3 = relu(la - e2);  r = relu(l*x)
                selu = r - e3
                """
                e2 = tmpp.tile([P, D], f32, tag="tmpd")
                nc.scalar.activation(e2[:], psum_ap,
                                     mybir.ActivationFunctionType.Exp,
                                     bias=ln_la[:])
                e3 = tmpp.tile([P, D], f32, tag="tmpd2")
                nc.scalar.activation(e3[:], e2[:],
                                     mybir.ActivationFunctionType.Relu,
                                     bias=la_c[:], scale=-1.0)
                r = tmpp.tile([P, D], f32, tag="tmpd3")
                nc.scalar.activation(r[:], psum_ap,
                                     mybir.ActivationFunctionType.Relu,
                                     scale=SELU_L)
                nc.vector.tensor_tensor(dst_ap, r[:], e3[:],
                                        mybir.AluOpType.subtract)

            gtile = {}
            LOOK = GT_LOOK
            gorder = [(0, b) for b in range(min(LOOK, NB))]
            for b in range(NB):
                gorder.append((1, b))
                if b + LOOK < NB:
                    gorder.append((0, b + LOOK))
            prev_inst = None
            gq = 0
            cnt_reg = nc.gpsimd.alloc_register("gcnt_reg")
            g10_inst = None
            for emit_rank, (h, g) in enumerate(gorder):
                sc = strm_base[h][g]
                n = CBH[g][h]
                gt = gtp.tile([P, max_grp, W2], fp8, tag="gt",
                              name=f"gt_{h}_{g}")
                if n > 0:
                    if emit_rank < GT_BUFS and n < max_grp:
                        # first ring pass leaves slices >= n uninitialized;
                        # zero them so later pad lanes never read NaN bits
                        nc.vector.memset(gt[:, n:max_grp, :], 0.0)
                    nidx = n * P
                    ld = nc.gpsimd.reg_load(
                        cnt_reg, gcnt_t[0:1, h * NB + g:h * NB + g + 1])
                    if prev_inst is not None:
                        tile.add_dep_helper(ld.ins, prev_inst, sync=False,
                                            reason="gather issue order")
                    gi = nc.gpsimd.dma_gather(
                        gt[:, 0:n, :], cc_out[h * HALF:(h + 1) * HALF, :],
                        idx_t[h][:, sc * 8:(sc + n) * 8],
                        num_idxs=nidx, num_idxs_reg=cnt_reg, elem_size=W2,
                        single_packet=False,
                        queue_num=gq % 4,
                    )
                    gq += 1
                    tile.add_dep_helper(gi.ins, ld.ins, sync=False,
                                        reason="count reg load order")
                    prev_inst = gi.ins
                    if (h, g) == (1, 0):
                        g10_inst = gi.ins
                gtile[(h, g)] = (gt, sc)

            def get_gtile(h, g):
                return gtile[(h, g)]

            first_oh = True
            for b in range(NB):
                rows = min(P, NSH - b * P)
                nch = CBH[b][0] + CBH[b][1]
                oht = ohp.tile([P, max_nch_b, P], fp8, tag="oh")
                ohl = nc.scalar.dma_start(oht[:, 0:nch, :],
                                          oh[:, ohbase[b]:ohbase[b] + nch, :])
                if first_oh:
                    # keep the 14MB one-hot stream off the wires until both
                    # AllGathers are done (block-0 matmuls need AG1 anyway)
                    tile.add_dep_helper(ohl.ins, g10_inst, sync=True,
                                        reason="delay oh stream")
                    first_oh = False
                pt = pa.tile([P, W2], f32, space="PSUM", tag="pa")
                nmm = 0
                tot = CBH[b][0] + CBH[b][1]
                if tot == 0:
                    nc.vector.memset(pt[:], 0.0)
                for h in range(2):
                    n = CBH[b][h]
                    if n == 0:
                        continue
                    gt, s = get_gtile(h, b)
                    base_oh = h * CBH[b][0]
                    for j in range(n):
                        c = strm_base[h][b] + j - s
                        nc.tensor.matmul(
                            pt[:], lhsT=oht[:, base_oh + j, :],
                            rhs=gt[:, c, :],
                            start=(nmm == 0), stop=(nmm == tot - 1))
                        nmm += 1

                # epilogue
                g1b = tmpp.tile([P, D], f32, tag="g1b")
                selu_into(g1b[:], pt[:, 0:D])
                aug = tmpp.tile([P, D], f32, tag="aug")
                selu_into(aug[:], pt[:, D:W2])
                # con partials
                nc.vector.tensor_tensor(accs[:, 0:D], accs[:, 0:D], aug[:],
                                        mybir.AluOpType.add)
                pr = tmpp.tile([P, D], f32, tag="pr")
                nc.vector.tensor_tensor(pr[:], aug[:], g1b[:],
                                        mybir.AluOpType.mult)
                nc.vector.tensor_tensor(accs[:, D:W2], accs[:, D:W2], pr[:],
                                        mybir.AluOpType.add)
                for t in range(DT):
                    ptr = pb.tile([P, P], f32, space="PSUM", tag="pb")
                    nc.tensor.transpose(ptr[:], g1b[:, t * P:(t + 1) * P],
                                        ident[:])
                    nc.vector.tensor_copy(gcn1T[:, t, b * P:(b + 1) * P], ptr[:])
                pl = pb.tile([P, K], f32, space="PSUM", tag="pb")
                for t in range(DT):
                    nc.tensor.matmul(pl[:], lhsT=gcn1T[:, t, b * P:(b + 1) * P],
                                     rhs=wtt_t[:, t, :],
                                     start=(t == 0), stop=(t == DT - 1))
                nmx = smallp.tile([P, 1], f32, tag="nmx")
                nc.vector.reduce_max(nmx[:], pl[:], axis=mybir.AxisListType.X,
                                     negate=True)
                ex = smallp.tile([P, K], f32, tag="ex")
                sm = smallp.tile([P, 1], f32, tag="sm")
                nc.scalar.activation(ex[:], pl[:],
                                     mybir.ActivationFunctionType.Exp,
                                     bias=nmx[:], accum_out=sm[:])
                rc = smallp.tile([P, 1], f32, tag="rc")
                nc.vector.reciprocal(rc[:], sm[:])
                sb = stagep.tile([P, K], f32, tag="sstage")
                nc.scalar.mul(sb[:], ex[:], rc[:])
                nc.sync.dma_start(s_out_d[b * P:(b + 1) * P, :], sb[:])

                # log-softmax stats for a column segment as soon as its
                # last block's gcn1T slice lands (keeps them off the tail)
                seg_ends = {NB0 - 1: (0, 0, NB0 * P),
                            NB - 2: (1, NB0 * P, (NB - 1) * P),
                            NB - 1: (2, (NB - 1) * P, NSH)}
                if b in seg_ends:
                    si, cl_, cr_ = seg_ends[b]
                    for t in range(DT):
                        nmt = smallp.tile([P, 1], f32, tag="nmt")
                        nc.vector.reduce_max(nmt[:], gcn1T[:, t, cl_:cr_],
                                             axis=mybir.AxisListType.X,
                                             negate=True)
                        nc.scalar.mul(
                            stats_p[:, si * 2 * DT + t:si * 2 * DT + t + 1],
                            nmt[:], -1.0)
                        sacc = stats_p[:, si * 2 * DT + DT + t:
                                       si * 2 * DT + DT + t + 1]
                        if cr_ - cl_ <= 17 * P:
                            exb = bigp.tile([P, 17 * P], bf16, tag="big")
                            nc.scalar.activation(
                                exb[:, 0:cr_ - cl_], gcn1T[:, t, cl_:cr_],
                                mybir.ActivationFunctionType.Exp, bias=nmt[:],
                                accum_out=sacc)
                        else:
                            mid = (cl_ + cr_) // 2
                            parts = smallp.tile([P, 2], f32, tag="parts")
                            for ci, (e0, e1) in enumerate(((cl_, mid),
                                                          (mid, cr_))):
                                exb = bigp.tile([P, 17 * P], bf16, tag="big")
                                nc.scalar.activation(
                                    exb[:, 0:e1 - e0], gcn1T[:, t, e0:e1],
                                    mybir.ActivationFunctionType.Exp,
                                    bias=nmt[:], accum_out=parts[:, ci:ci + 1])
                            nc.vector.tensor_tensor(sacc, parts[:, 0:1],
                                                    parts[:, 1:2],
                                                    mybir.AluOpType.add)

            ones = persist.tile([P, 1], f32, tag="ones")
            nc.vector.memset(ones[:], 1.0)
            pv = pa.tile([P, VLEN], f32, space="PSUM", tag="pa")
            nc.tensor.matmul(pv[0:1, :], lhsT=ones[:], rhs=accs[:],
                             start=True, stop=True)
            sv = svp.tile([1, VLEN], f32, tag="sv")
            nc.vector.tensor_copy(sv[:], pv[0:1, :])
            nc.sync.dma_start(stats_v_d[:], sv[:])
            nc.sync.dma_start(stats_p_d[:], stats_p[:])

    nc.compile()
    return nc


# --------------------------------------------------------------------------
# host-side combine of per-core partials
# --------------------------------------------------------------------------

def combine(results, cfg, graph_row, graph_col, graph_vals):
    N, D, K = cfg["N"], cfg["D"], cfg["K"]
    NSH = N // NC
    DT = D // P
    E = float(graph_row.shape[0])

    m, s = [], []
    colsum_aug = np.zeros(D)
    dot = 0.0
    S_full = np.zeros((N, K))
    for c in range(NC):
        sp = np.asarray(results[c]["stats_p"], dtype=np.float64)
        svv = np.asarray(results[c]["stats_v"], dtype=np.float64).reshape(-1)
        for si in range(3):
            m.append(np.concatenate(
                [sp[:, si * 2 * DT + t] for t in range(DT)]))
            s.append(np.concatenate(
                [sp[:, si * 2 * DT + DT + t] for t in range(DT)]))
        colsum_aug += svv[0:D]
        dot += svv[D:2 * D].sum()
        S_full[c * NSH:(c + 1) * NSH] = \
            np.asarray(results[c]["s_out"], dtype=np.float64)[:NSH]
    m = np.stack(m)
    s = np.stack(s)
    M = m.max(axis=0)
    Sg = (np.exp(m - M) * s).sum(axis=0)
    logZ = M + np.log(Sg)

    row = np.asarray(graph_row).astype(np.int64)
    col = np.asarray(graph_col).astype(np.int64)
    av = np.asarray(graph_vals).astype(np.float64)
    deg = np.bincount(col, weights=av, minlength=N).astype(np.float64)

    trace_gp = np.einsum('e,ek,ek->', av, S_full[row], S_full[col])
    nl = S_full.T @ deg
    clsz = S_full.sum(axis=0)

    spectral = -(trace_gp - (nl ** 2).sum() / (2.0 * E)) / (2.0 * E)
    cluster = (np.linalg.norm(clsz) / N * math.sqrt(K) - 1.0) * CLUSTER_REG
    con = -(dot - (logZ * colsum_aug).sum()) / D
    return spectral + cluster + CON_REG * con


# --------------------------------------------------------------------------
# entry point
# --------------------------------------------------------------------------

_BUILD_CACHE = {}


def kernel(features, aug_features, graph_row, graph_col, graph_vals, gn_vals,
           lbl, dense_graph, W1, b1, Wt, bt, _cfg=None, _trace=False):
    cfg = _cfg or FULL
    in_maps, meta = prep(features, aug_features, graph_row, graph_col,
                         gn_vals, W1, Wt, cfg)
    key = tuple(sorted((k, str(v)) for k, v in meta.items()))
    if key not in _BUILD_CACHE:
        _BUILD_CACHE[key] = build(meta)
    nc = _BUILD_CACHE[key]
    res = bass_utils.run_bass_kernel_spmd(nc, in_maps, core_ids=list(range(NC)),
                                          trace=_trace)
    loss = combine(res.results, cfg, graph_row, graph_col, graph_vals)
    out = np.array(loss, dtype=np.float32)
    if _trace:
        return out, res
    return out

